# revision 9
# baseline (speedup 1.0000x reference)
"""Causal single-head attention (B=4, S=2048, d=1024) on 8 trn2 NeuronCores.

Sharding: core c -> batch c//2, query-parity c%2. Queries of one batch are
split by even/odd 128-row blocks (interleaved so causal work balances);
every core runs the IDENTICAL program -- the host gathers each core's query
rows into a dense x_q input, and two per-core [128,512] additive masks
encode the causal boundary (even-parity cores get different masks than
odd-parity cores). Each core redundantly computes K and V for its batch.

Per-core pipeline (all matmuls float32r: full PE rate at N>=256, ~1.5e-4
relative error; every matmul is an LDWEIGHTS+MATMUL pair, so large moving
dims amortize the ~225ns weight-load):
  P1a: PE-transpose x_q chunks -> X_q^T; Q^T = Wq^T X_q^T         (SBUF)
  P1b: PE-transpose x chunks -> X^T; V = X Wv -> DRAM scratch
       (V interleaved with transposes to keep the PE HAM warm);
       K^T = Wk^T X^T                                              (SBUF)
  P2:  for each 512-key block j (V streamed back), query block i >= 2j:
       scores = Q_i^T.T @ K_j [128,512]; boundary block += mask0/1;
       E = exp(scores/32) on ScalarE with fused row-sum accum_out;
       E^T via 4 PE transposes batched into one PSUM tile; AV
       accumulated over the 4 key sub-tiles in PSUM, then into SBUF
       out_acc.  Finally out_acc *= 1/l, DMA to y.
"""

import sys

import numpy as np

if "/opt/trn_rl_repo" not in sys.path:
    sys.path.insert(0, "/opt/trn_rl_repo")

B = 4
S = 2048
D = 1024
NB = 8  # query blocks of 128 per core
KH = 8  # 128-row tiles along d_in / d_out
NEG = -1.0e9
SCALE = float(D) ** -0.5  # 1/32

_CACHE = {}
LAST_RESULT = None


def _build_nc():
    import contextlib

    import concourse.bacc as bacc
    import concourse.mybir as mybir
    import concourse.tile as tile

    F32 = mybir.dt.float32
    F32R = mybir.dt.float32r

    nc = bacc.Bacc(None, target_bir_lowering=False)

    x_T = nc.dram_tensor("x_T", [D, S], F32, kind="ExternalInput")
    x_qT = nc.dram_tensor("x_qT", [D, NB * 128], F32, kind="ExternalInput")
    wq = nc.dram_tensor("wq", [D, D], F32, kind="ExternalInput")
    wk = nc.dram_tensor("wk", [D, D], F32, kind="ExternalInput")
    wv = nc.dram_tensor("wv", [D, D], F32, kind="ExternalInput")
    mask = nc.dram_tensor("mask", [2, 128, 512], F32, kind="ExternalInput")
    ident_in = nc.dram_tensor("ident", [128, 128], F32, kind="ExternalInput")
    y = nc.dram_tensor("y", [NB * 128, D], F32, kind="ExternalOutput")
    v_dram = nc.dram_tensor("v_scratch", [S, D], F32)  # Internal scratch

    # DRAM views with the 128-partition tiling of the d_in axis
    wq_t = wq.rearrange("(kh p) n -> p kh n", p=128)
    wk_t = wk.rearrange("(kh p) n -> p kh n", p=128)
    wv_t = wv.rearrange("(kh p) n -> p kh n", p=128)

    with tile.TileContext(nc) as tc:
        with contextlib.ExitStack() as ctx:
            persist = ctx.enter_context(tc.tile_pool(name="persist", bufs=1))

            ident = persist.tile([128, 128], F32)
            nc.sync.dma_start(out=ident, in_=ident_in[:, :])
            mask_sb = persist.tile([128, 2, 512], F32)
            q_T = persist.tile([128, KH, NB * 128], F32R)  # [d_lo, d_hi, sq]
            k_T = persist.tile([128, KH, S], F32R)  # [d_lo, d_hi, sk]
            l_acc = persist.tile([128, NB], F32)

            xT_view = x_T.rearrange("(kh p) s -> p kh s", p=128)
            xqT_view = x_qT.rearrange("(kh p) s -> p kh s", p=128)

            # ---------------- Phase 1: projections ----------------
            with (
                tc.tile_pool(name="wpool", bufs=2) as wpool,
                tc.tile_pool(name="xT", bufs=2) as xT_pool,
                tc.tile_pool(name="vstage", bufs=2) as vstage_pool,
                tc.tile_pool(name="mmps", bufs=4, space="PSUM") as mmps_pool,
            ):
                # --- P1a: Q^T from x_q (2 chunks of 512 rows) ---
                # x DMAs go on the sync (SP) HWDGE queue, weight DMAs on the
                # scalar (ACT) HWDGE queue so neither blocks the other.
                # Weights stream in per-kh slices so the k=0 matmuls can
                # start as soon as the first 512KB lands.
                wq_sb = wpool.tile([128, KH, D], F32R, tag="w")
                for k in range(KH):
                    nc.scalar.dma_start(
                        out=wq_sb[:, k, :], in_=wq_t[:, k, :].bitcast(F32R)
                    )
                wk_sb = wpool.tile([128, KH, D], F32R, tag="w")
                for k in range(KH):
                    nc.scalar.dma_start(
                        out=wk_sb[:, k, :], in_=wk_t[:, k, :].bitcast(F32R)
                    )

                for strip in range(2):  # 512 query rows each
                    xTq = xT_pool.tile([128, KH, 512], F32R, tag="xT")
                    nc.sync.dma_start(
                        out=xTq,
                        in_=xqT_view[:, :, strip * 512 : (strip + 1) * 512].bitcast(
                            F32R
                        ),
                    )
                    for h in range(KH):
                        qps = mmps_pool.tile([128, 512], F32, tag="mm")
                        for k in range(KH):
                            nc.tensor.matmul(
                                qps,
                                wq_sb[:, k, h * 128 : (h + 1) * 128],
                                xTq[:, k, :],
                                start=(k == 0),
                                stop=(k == KH - 1),
                            )
                        nc.vector.tensor_copy(
                            out=q_T[:, h, strip * 512 : (strip + 1) * 512],
                            in_=qps,
                        )

                # --- P1b: V (DRAM scratch) and K^T (SBUF) from x ---
                wv_sb = wpool.tile([128, KH, D], F32R, tag="w")
                for k in range(KH):
                    nc.scalar.dma_start(
                        out=wv_sb[:, k, :], in_=wv_t[:, k, :].bitcast(F32R)
                    )

                for chunk in range(4):  # 512 seq rows each
                    xT = xT_pool.tile([128, KH, 512], F32R, tag="xT")
                    nc.sync.dma_start(
                        out=xT,
                        in_=xT_view[:, :, chunk * 512 : (chunk + 1) * 512].bitcast(
                            F32R
                        ),
                    )
                    for t in range(4):
                        r0 = chunk * 512 + t * 128
                        for dh in range(2):
                            vps = mmps_pool.tile([128, 512], F32, tag="mm")
                            for k in range(KH):
                                nc.tensor.matmul(
                                    vps,
                                    xT[:, k, t * 128 : (t + 1) * 128],
                                    wv_sb[:, k, dh * 512 : (dh + 1) * 512],
                                    start=(k == 0),
                                    stop=(k == KH - 1),
                                )
                            vstage = vstage_pool.tile([128, 512], F32, tag="vs")
                            nc.scalar.copy(out=vstage, in_=vps)
                            nc.scalar.dma_start(
                                out=v_dram[r0 : r0 + 128, dh * 512 : (dh + 1) * 512],
                                in_=vstage,
                            )
                    for h in range(KH):
                        kps = mmps_pool.tile([128, 512], F32, tag="mm")
                        for k in range(KH):
                            nc.tensor.matmul(
                                kps,
                                wk_sb[:, k, h * 128 : (h + 1) * 128],
                                xT[:, k, :],
                                start=(k == 0),
                                stop=(k == KH - 1),
                            )
                        nc.vector.tensor_copy(
                            out=k_T[:, h, chunk * 512 : (chunk + 1) * 512], in_=kps
                        )

            # ---------------- Phase 2: attention ----------------
            v_view = v_dram.rearrange("(a p) n -> p a n", p=128)  # a = s//128
            with (
                tc.tile_pool(name="oacc", bufs=1) as oacc_pool,
                tc.tile_pool(name="vsb", bufs=2) as vsb_pool,
                tc.tile_pool(name="esb", bufs=3) as esb_pool,
                tc.tile_pool(name="etsb", bufs=3) as etsb_pool,
                tc.tile_pool(name="lsb", bufs=4) as lsb_pool,
                tc.tile_pool(name="sps", bufs=2, space="PSUM") as sps_pool,
                tc.tile_pool(name="etps", bufs=2, space="PSUM") as etps_pool,
                tc.tile_pool(name="avps", bufs=2, space="PSUM") as avps_pool,
            ):
                out_acc = oacc_pool.tile([128, NB, D], F32)
                nc.sync.dma_start(out=mask_sb, in_=mask.rearrange("m p n -> p m n"))

                for j in range(4):  # 512-key blocks
                    v_sb = vsb_pool.tile([128, 4, D], F32R, tag="v")
                    nc.sync.dma_start(
                        out=v_sb, in_=v_view[:, 4 * j : 4 * j + 4, :].bitcast(F32R)
                    )
                    for i in range(2 * j, NB):  # query blocks seeing key block j
                        diag = (i // 2) == j
                        sps = sps_pool.tile([128, 512], F32, tag="s")
                        for k in range(KH):
                            nc.tensor.matmul(
                                sps,
                                q_T[:, k, i * 128 : (i + 1) * 128],
                                k_T[:, k, j * 512 : (j + 1) * 512],
                                start=(k == 0),
                                stop=(k == KH - 1),
                            )
                        if diag:  # causal boundary block
                            nc.vector.tensor_add(
                                out=sps, in0=sps, in1=mask_sb[:, i % 2, :]
                            )
                        e_sb = esb_pool.tile([128, 512], F32, tag="e")
                        lpart = lsb_pool.tile([128, 1], F32, tag="l")
                        nc.scalar.activation(
                            out=e_sb,
                            in_=sps,
                            func=mybir.ActivationFunctionType.Exp,
                            scale=SCALE,
                            accum_out=lpart,
                        )
                        if j == 0:
                            nc.vector.tensor_copy(out=l_acc[:, i : i + 1], in_=lpart)
                        else:
                            nc.vector.tensor_add(
                                out=l_acc[:, i : i + 1],
                                in0=l_acc[:, i : i + 1],
                                in1=lpart,
                            )
                        # E^T: 4 PE transposes into one PSUM tile, one copy out
                        etp = etps_pool.tile([128, 512], F32, tag="et")
                        for jj in range(4):
                            nc.tensor.transpose(
                                etp[:, jj * 128 : (jj + 1) * 128],
                                e_sb[:, jj * 128 : (jj + 1) * 128],
                                ident,
                            )
                        et = etsb_pool.tile([128, 512], F32R, tag="ets")
                        nc.scalar.copy(out=et, in_=etp)
                        avps = avps_pool.tile([128, D], F32, tag="av")
                        for jj in range(4):
                            for dh in range(2):
                                nc.tensor.matmul(
                                    avps[:, dh * 512 : (dh + 1) * 512],
                                    et[:, jj * 128 : (jj + 1) * 128],
                                    v_sb[:, jj, dh * 512 : (dh + 1) * 512],
                                    start=(jj == 0),
                                    stop=(jj == 3),
                                )
                        if j == 0:
                            nc.vector.tensor_copy(out=out_acc[:, i, :], in_=avps)
                        else:
                            nc.vector.tensor_add(
                                out=out_acc[:, i, :], in0=out_acc[:, i, :], in1=avps
                            )
                        if diag:
                            # block i is complete: normalize and write out now
                            rinv = lsb_pool.tile([128, 1], F32, tag="r")
                            nc.vector.reciprocal(out=rinv, in_=l_acc[:, i : i + 1])
                            nc.vector.tensor_scalar_mul(
                                out=out_acc[:, i, :],
                                in0=out_acc[:, i, :],
                                scalar1=rinv,
                            )
                            nc.scalar.dma_start(
                                out=y[i * 128 : (i + 1) * 128, :],
                                in_=out_acc[:, i, :],
                            )

    return nc


def _get_nc(finalize=True):
    key = "nc_fin" if finalize else "nc_raw"
    if key not in _CACHE:
        nc = _build_nc()
        if finalize:
            nc.finalize()
        _CACHE[key] = nc
    return _CACHE[key]


def make_in_maps(x, Wq, Wk, Wv):
    ident = np.eye(128, dtype=np.float32)
    p = np.arange(128)[:, None]
    c = np.arange(512)[None, :]
    in_maps = []
    for core in range(8):
        b, par = core // 2, core % 2
        # mask[0]: boundary block for even local i; mask[1]: odd local i
        m0 = np.where(c <= p + par * 128, 0.0, NEG).astype(np.float32)
        m1 = np.where(c <= 256 + par * 128 + p, 0.0, NEG).astype(np.float32)
        xb = np.asarray(x[b], dtype=np.float32)
        xq = xb.reshape(16, 128, D)[par::2].reshape(NB * 128, D)
        in_maps.append(
            {
                "x_T": np.ascontiguousarray(xb.T),
                "x_qT": np.ascontiguousarray(xq.T),
                "wq": np.ascontiguousarray(Wq, dtype=np.float32),
                "wk": np.ascontiguousarray(Wk, dtype=np.float32),
                "wv": np.ascontiguousarray(Wv, dtype=np.float32),
                "mask": np.stack([m0, m1]),
                "ident": ident,
            }
        )
    return in_maps


def assemble_out(results):
    out = np.empty((B, S, D), dtype=np.float32)
    o4 = out.reshape(B, 16, 128, D)
    for core in range(8):
        b, par = core // 2, core % 2
        o4[b, par::2] = results[core]["y"].reshape(NB, 128, D)
    return out


def kernel(x, Wq, Wk, Wv):
    global LAST_RESULT
    from concourse.bass_utils import run_bass_kernel_spmd

    nc = _get_nc(finalize=True)
    in_maps = make_in_maps(x, Wq, Wk, Wv)
    res = run_bass_kernel_spmd(nc, in_maps, core_ids=list(range(8)))
    LAST_RESULT = res
    return assemble_out(res.results)


# revision 10
# speedup vs baseline: 1.1665x; 1.1665x over previous
"""Causal single-head attention (B=4, S=2048, d=1024) on 8 trn2 NeuronCores.

Sharding: core c -> batch c//2, query-parity c%2. Queries of one batch are
split by even/odd 128-row blocks (interleaved so causal work balances);
every core runs the IDENTICAL program -- the host gathers each core's query
rows into a dense x_q input, and two per-core [128,512] additive masks
encode the causal boundary (even-parity cores get different masks than
odd-parity cores). Each core redundantly computes K and V for its batch.

Per-core pipeline (all matmuls float32r: full PE rate at N>=256, ~1.5e-4
relative error; every matmul is an LDWEIGHTS+MATMUL pair, so large moving
dims amortize the ~225ns weight-load):
  P1a: PE-transpose x_q chunks -> X_q^T; Q^T = Wq^T X_q^T         (SBUF)
  P1b: PE-transpose x chunks -> X^T; V = X Wv -> DRAM scratch
       (V interleaved with transposes to keep the PE HAM warm);
       K^T = Wk^T X^T                                              (SBUF)
  P2:  for each 512-key block j (V streamed back), query block i >= 2j:
       scores = Q_i^T.T @ K_j [128,512]; boundary block += mask0/1;
       E = exp(scores/32) on ScalarE with fused row-sum accum_out;
       E^T via 4 PE transposes batched into one PSUM tile; AV
       accumulated over the 4 key sub-tiles in PSUM, then into SBUF
       out_acc.  Finally out_acc *= 1/l, DMA to y.
"""

import sys

import numpy as np

if "/opt/trn_rl_repo" not in sys.path:
    sys.path.insert(0, "/opt/trn_rl_repo")

B = 4
S = 2048
D = 1024
NB = 8  # query blocks of 128 per core
KH = 8  # 128-row tiles along d_in / d_out
NEG = -1.0e9
SCALE = float(D) ** -0.5  # 1/32

_CACHE = {}
LAST_RESULT = None


def _build_nc():
    import contextlib

    import concourse.bacc as bacc
    import concourse.mybir as mybir
    import concourse.tile as tile

    F32 = mybir.dt.float32
    F32R = mybir.dt.float32r

    nc = bacc.Bacc(None, target_bir_lowering=False)

    x_T = nc.dram_tensor("x_T", [D, S], F32, kind="ExternalInput")
    x_qT = nc.dram_tensor("x_qT", [D, NB * 128], F32, kind="ExternalInput")
    wq = nc.dram_tensor("wq", [D, D], F32, kind="ExternalInput")
    wk = nc.dram_tensor("wk", [D, D], F32, kind="ExternalInput")
    wv = nc.dram_tensor("wv", [D, D], F32, kind="ExternalInput")
    mask = nc.dram_tensor("mask", [2, 128, 512], F32, kind="ExternalInput")
    ident_in = nc.dram_tensor("ident", [128, 128], F32, kind="ExternalInput")
    y = nc.dram_tensor("y", [NB * 128, D], F32, kind="ExternalOutput")
    v_dram = nc.dram_tensor("v_scratch", [S, D], F32)  # Internal scratch

    # DRAM views with the 128-partition tiling of the d_in axis
    wq_t = wq.rearrange("(kh p) n -> p kh n", p=128)
    wk_t = wk.rearrange("(kh p) n -> p kh n", p=128)
    wv_t = wv.rearrange("(kh p) n -> p kh n", p=128)

    with tile.TileContext(nc) as tc:
        with contextlib.ExitStack() as ctx:
            persist = ctx.enter_context(tc.tile_pool(name="persist", bufs=1))

            ident = persist.tile([128, 128], F32)
            nc.sync.dma_start(out=ident, in_=ident_in[:, :])
            mask_sb = persist.tile([128, 2, 512], F32)
            q_T = persist.tile([128, KH, NB * 128], F32R)  # [d_lo, d_hi, sq]
            k_T = persist.tile([128, KH, S], F32R)  # [d_lo, d_hi, sk]
            l_acc = persist.tile([128, NB], F32)

            xT_view = x_T.rearrange("(kh p) s -> p kh s", p=128)
            xqT_view = x_qT.rearrange("(kh p) s -> p kh s", p=128)

            # ---------------- Phase 1: projections ----------------
            with (
                tc.tile_pool(name="wpool", bufs=2) as wpool,
                tc.tile_pool(name="xT", bufs=2) as xT_pool,
                tc.tile_pool(name="vstage", bufs=2) as vstage_pool,
                tc.tile_pool(name="mmps", bufs=4, space="PSUM") as mmps_pool,
            ):
                # --- P1a: Q^T from x_q (2 chunks of 512 rows) ---
                # x DMAs go on the sync (SP) HWDGE queue, weight DMAs on the
                # scalar (ACT) HWDGE queue so neither blocks the other.
                # Weights stream in per-kh slices so the k=0 matmuls can
                # start as soon as the first 512KB lands.
                wq_sb = wpool.tile([128, KH, D], F32R, tag="w")
                for k in range(KH):
                    nc.scalar.dma_start(
                        out=wq_sb[:, k, :], in_=wq_t[:, k, :].bitcast(F32R)
                    )
                wk_sb = wpool.tile([128, KH, D], F32R, tag="w")
                for k in range(KH):
                    nc.scalar.dma_start(
                        out=wk_sb[:, k, :], in_=wk_t[:, k, :].bitcast(F32R)
                    )

                for strip in range(2):  # 512 query rows each
                    xTq = xT_pool.tile([128, KH, 512], F32R, tag="xT")
                    for k in range(KH):
                        nc.sync.dma_start(
                            out=xTq[:, k, :],
                            in_=xqT_view[
                                :, k, strip * 512 : (strip + 1) * 512
                            ].bitcast(F32R),
                        )
                    for h in range(KH):
                        qps = mmps_pool.tile([128, 512], F32, tag="mm")
                        for k in range(KH):
                            nc.tensor.matmul(
                                qps,
                                wq_sb[:, k, h * 128 : (h + 1) * 128],
                                xTq[:, k, :],
                                start=(k == 0),
                                stop=(k == KH - 1),
                            )
                        nc.vector.tensor_copy(
                            out=q_T[:, h, strip * 512 : (strip + 1) * 512],
                            in_=qps,
                        )

                # --- P1b: V (DRAM scratch) and K^T (SBUF) from x ---
                wv_sb = wpool.tile([128, KH, D], F32R, tag="w")
                for k in range(KH):
                    nc.scalar.dma_start(
                        out=wv_sb[:, k, :], in_=wv_t[:, k, :].bitcast(F32R)
                    )

                for chunk in range(4):  # 512 seq rows each
                    xT = xT_pool.tile([128, KH, 512], F32R, tag="xT")
                    for k in range(KH):
                        nc.sync.dma_start(
                            out=xT[:, k, :],
                            in_=xT_view[
                                :, k, chunk * 512 : (chunk + 1) * 512
                            ].bitcast(F32R),
                        )
                    for t in range(4):
                        r0 = chunk * 512 + t * 128
                        for dh in range(2):
                            vps = mmps_pool.tile([128, 512], F32, tag="mm")
                            for k in range(KH):
                                nc.tensor.matmul(
                                    vps,
                                    xT[:, k, t * 128 : (t + 1) * 128],
                                    wv_sb[:, k, dh * 512 : (dh + 1) * 512],
                                    start=(k == 0),
                                    stop=(k == KH - 1),
                                )
                            vstage = vstage_pool.tile([128, 512], F32, tag="vs")
                            nc.scalar.copy(out=vstage, in_=vps)
                            nc.gpsimd.dma_start(
                                out=v_dram[r0 : r0 + 128, dh * 512 : (dh + 1) * 512],
                                in_=vstage,
                            )
                    for h in range(KH):
                        kps = mmps_pool.tile([128, 512], F32, tag="mm")
                        for k in range(KH):
                            nc.tensor.matmul(
                                kps,
                                wk_sb[:, k, h * 128 : (h + 1) * 128],
                                xT[:, k, :],
                                start=(k == 0),
                                stop=(k == KH - 1),
                            )
                        nc.vector.tensor_copy(
                            out=k_T[:, h, chunk * 512 : (chunk + 1) * 512], in_=kps
                        )

            # ---------------- Phase 2: attention ----------------
            v_view = v_dram.rearrange("(a p) n -> p a n", p=128)  # a = s//128
            with (
                tc.tile_pool(name="oacc", bufs=1) as oacc_pool,
                tc.tile_pool(name="vsb", bufs=2) as vsb_pool,
                tc.tile_pool(name="esb", bufs=3) as esb_pool,
                tc.tile_pool(name="etsb", bufs=3) as etsb_pool,
                tc.tile_pool(name="lsb", bufs=4) as lsb_pool,
                tc.tile_pool(name="sps", bufs=2, space="PSUM") as sps_pool,
                tc.tile_pool(name="etps", bufs=2, space="PSUM") as etps_pool,
                tc.tile_pool(name="avps", bufs=2, space="PSUM") as avps_pool,
            ):
                out_acc = oacc_pool.tile([128, NB, D], F32)
                nc.sync.dma_start(out=mask_sb, in_=mask.rearrange("m p n -> p m n"))

                for j in range(4):  # 512-key blocks
                    v_sb = vsb_pool.tile([128, 4, D], F32R, tag="v")
                    nc.sync.dma_start(
                        out=v_sb, in_=v_view[:, 4 * j : 4 * j + 4, :].bitcast(F32R)
                    )
                    for i in range(2 * j, NB):  # query blocks seeing key block j
                        diag = (i // 2) == j
                        sps = sps_pool.tile([128, 512], F32, tag="s")
                        for k in range(KH):
                            nc.tensor.matmul(
                                sps,
                                q_T[:, k, i * 128 : (i + 1) * 128],
                                k_T[:, k, j * 512 : (j + 1) * 512],
                                start=(k == 0),
                                stop=(k == KH - 1),
                            )
                        if diag:  # causal boundary block
                            nc.vector.tensor_add(
                                out=sps, in0=sps, in1=mask_sb[:, i % 2, :]
                            )
                        e_sb = esb_pool.tile([128, 512], F32, tag="e")
                        lpart = lsb_pool.tile([128, 1], F32, tag="l")
                        nc.scalar.activation(
                            out=e_sb,
                            in_=sps,
                            func=mybir.ActivationFunctionType.Exp,
                            scale=SCALE,
                            accum_out=lpart,
                        )
                        if j == 0:
                            nc.vector.tensor_copy(out=l_acc[:, i : i + 1], in_=lpart)
                        else:
                            nc.vector.tensor_add(
                                out=l_acc[:, i : i + 1],
                                in0=l_acc[:, i : i + 1],
                                in1=lpart,
                            )
                        # E^T: 4 PE transposes into one PSUM tile, one copy out
                        etp = etps_pool.tile([128, 512], F32, tag="et")
                        for jj in range(4):
                            nc.tensor.transpose(
                                etp[:, jj * 128 : (jj + 1) * 128],
                                e_sb[:, jj * 128 : (jj + 1) * 128],
                                ident,
                            )
                        et = etsb_pool.tile([128, 512], F32R, tag="ets")
                        nc.vector.tensor_copy(out=et, in_=etp)
                        avps = avps_pool.tile([128, D], F32, tag="av")
                        for jj in range(4):
                            for dh in range(2):
                                nc.tensor.matmul(
                                    avps[:, dh * 512 : (dh + 1) * 512],
                                    et[:, jj * 128 : (jj + 1) * 128],
                                    v_sb[:, jj, dh * 512 : (dh + 1) * 512],
                                    start=(jj == 0),
                                    stop=(jj == 3),
                                )
                        if j == 0:
                            nc.vector.tensor_copy(out=out_acc[:, i, :], in_=avps)
                        else:
                            nc.vector.tensor_add(
                                out=out_acc[:, i, :], in0=out_acc[:, i, :], in1=avps
                            )
                        if diag:
                            # block i is complete: normalize and write out now
                            rinv = lsb_pool.tile([128, 1], F32, tag="r")
                            nc.vector.reciprocal(out=rinv, in_=l_acc[:, i : i + 1])
                            nc.vector.tensor_scalar_mul(
                                out=out_acc[:, i, :],
                                in0=out_acc[:, i, :],
                                scalar1=rinv,
                            )
                            nc.gpsimd.dma_start(
                                out=y[i * 128 : (i + 1) * 128, :],
                                in_=out_acc[:, i, :],
                            )

    return nc


def _get_nc(finalize=True):
    key = "nc_fin" if finalize else "nc_raw"
    if key not in _CACHE:
        nc = _build_nc()
        if finalize:
            nc.finalize()
        _CACHE[key] = nc
    return _CACHE[key]


def make_in_maps(x, Wq, Wk, Wv):
    ident = np.eye(128, dtype=np.float32)
    p = np.arange(128)[:, None]
    c = np.arange(512)[None, :]
    in_maps = []
    for core in range(8):
        b, par = core // 2, core % 2
        # mask[0]: boundary block for even local i; mask[1]: odd local i
        m0 = np.where(c <= p + par * 128, 0.0, NEG).astype(np.float32)
        m1 = np.where(c <= 256 + par * 128 + p, 0.0, NEG).astype(np.float32)
        xb = np.asarray(x[b], dtype=np.float32)
        xq = xb.reshape(16, 128, D)[par::2].reshape(NB * 128, D)
        in_maps.append(
            {
                "x_T": np.ascontiguousarray(xb.T),
                "x_qT": np.ascontiguousarray(xq.T),
                "wq": np.ascontiguousarray(Wq, dtype=np.float32),
                "wk": np.ascontiguousarray(Wk, dtype=np.float32),
                "wv": np.ascontiguousarray(Wv, dtype=np.float32),
                "mask": np.stack([m0, m1]),
                "ident": ident,
            }
        )
    return in_maps


def assemble_out(results):
    out = np.empty((B, S, D), dtype=np.float32)
    o4 = out.reshape(B, 16, 128, D)
    for core in range(8):
        b, par = core // 2, core % 2
        o4[b, par::2] = results[core]["y"].reshape(NB, 128, D)
    return out


def kernel(x, Wq, Wk, Wv):
    global LAST_RESULT
    from concourse.bass_utils import run_bass_kernel_spmd

    nc = _get_nc(finalize=True)
    in_maps = make_in_maps(x, Wq, Wk, Wv)
    res = run_bass_kernel_spmd(nc, in_maps, core_ids=list(range(8)))
    LAST_RESULT = res
    return assemble_out(res.results)


# revision 13
# speedup vs baseline: 1.1761x; 1.0082x over previous
"""Causal single-head attention (B=4, S=2048, d=1024) on 8 trn2 NeuronCores.

Sharding: core c -> batch c//2, query-parity c%2. Queries of one batch are
split by even/odd 128-row blocks (interleaved so causal work balances);
every core runs the IDENTICAL program -- the host gathers each core's query
rows into a dense x_q input, and two per-core [128,512] additive masks
encode the causal boundary (even-parity cores get different masks than
odd-parity cores). Each core redundantly computes K and V for its batch.

Per-core pipeline (all matmuls float32r: full PE rate at N>=256, ~1.5e-4
relative error; every matmul is an LDWEIGHTS+MATMUL pair, so large moving
dims amortize the ~225ns weight-load):
  P1a: PE-transpose x_q chunks -> X_q^T; Q^T = Wq^T X_q^T         (SBUF)
  P1b: PE-transpose x chunks -> X^T; V = X Wv -> DRAM scratch
       (V interleaved with transposes to keep the PE HAM warm);
       K^T = Wk^T X^T                                              (SBUF)
  P2:  for each 512-key block j (V streamed back), query block i >= 2j:
       scores = Q_i^T.T @ K_j [128,512]; boundary block += mask0/1;
       E = exp(scores/32) on ScalarE with fused row-sum accum_out;
       E^T via 4 PE transposes batched into one PSUM tile; AV
       accumulated over the 4 key sub-tiles in PSUM, then into SBUF
       out_acc.  Finally out_acc *= 1/l, DMA to y.
"""

import sys

import numpy as np

if "/opt/trn_rl_repo" not in sys.path:
    sys.path.insert(0, "/opt/trn_rl_repo")

B = 4
S = 2048
D = 1024
NB = 8  # query blocks of 128 per core
KH = 8  # 128-row tiles along d_in / d_out
NEG = -1.0e9
SCALE = float(D) ** -0.5  # 1/32

_CACHE = {}
LAST_RESULT = None


def _build_nc():
    import contextlib

    import concourse.bacc as bacc
    import concourse.mybir as mybir
    import concourse.tile as tile

    F32 = mybir.dt.float32
    F32R = mybir.dt.float32r

    nc = bacc.Bacc(None, target_bir_lowering=False)

    x_T = nc.dram_tensor("x_T", [D, S], F32, kind="ExternalInput")
    x_qT = nc.dram_tensor("x_qT", [D, NB * 128], F32, kind="ExternalInput")
    wq = nc.dram_tensor("wq", [D, D], F32, kind="ExternalInput")
    wk = nc.dram_tensor("wk", [D, D], F32, kind="ExternalInput")
    wv = nc.dram_tensor("wv", [D, D], F32, kind="ExternalInput")
    mask = nc.dram_tensor("mask", [2, 128, 512], F32, kind="ExternalInput")
    ident_in = nc.dram_tensor("ident", [128, 128], F32, kind="ExternalInput")
    y = nc.dram_tensor("y", [NB * 128, D], F32, kind="ExternalOutput")
    v_dram = nc.dram_tensor("v_scratch", [S, D], F32)  # Internal scratch

    # DRAM views with the 128-partition tiling of the d_in axis
    wq_t = wq.rearrange("(kh p) n -> p kh n", p=128)
    wk_t = wk.rearrange("(kh p) n -> p kh n", p=128)
    wv_t = wv.rearrange("(kh p) n -> p kh n", p=128)

    with tile.TileContext(nc) as tc:
        with contextlib.ExitStack() as ctx:
            persist = ctx.enter_context(tc.tile_pool(name="persist", bufs=1))

            ident = persist.tile([128, 128], F32)
            nc.sync.dma_start(out=ident, in_=ident_in[:, :])
            mask_sb = persist.tile([128, 2, 512], F32)
            q_T = persist.tile([128, KH, NB * 128], F32R)  # [d_lo, d_hi, sq]
            k_T = persist.tile([128, KH, S], F32R)  # [d_lo, d_hi, sk]
            l_acc = persist.tile([128, NB], F32)

            xT_view = x_T.rearrange("(kh p) s -> p kh s", p=128)
            xqT_view = x_qT.rearrange("(kh p) s -> p kh s", p=128)

            # ---------------- Phase 1: projections ----------------
            with (
                tc.tile_pool(name="wpool", bufs=2) as wpool,
                tc.tile_pool(name="xT", bufs=2) as xT_pool,
                tc.tile_pool(name="vstage", bufs=2) as vstage_pool,
                tc.tile_pool(name="mmps", bufs=4, space="PSUM") as mmps_pool,
            ):
                # --- P1a: Q^T from x_q (2 chunks of 512 rows) ---
                # x DMAs go on the sync (SP) HWDGE queue, weight DMAs on the
                # scalar (ACT) HWDGE queue so neither blocks the other.
                # Weights stream in per-kh slices so the k=0 matmuls can
                # start as soon as the first 512KB lands.
                wk_sb = wpool.tile([128, KH, D], F32R, tag="w")
                for k in range(KH):
                    nc.scalar.dma_start(
                        out=wk_sb[:, k, :], in_=wk_t[:, k, :].bitcast(F32R)
                    )
                wv_sb = wpool.tile([128, KH, D], F32R, tag="w")
                for k in range(KH):
                    nc.scalar.dma_start(
                        out=wv_sb[:, k, :], in_=wv_t[:, k, :].bitcast(F32R)
                    )
                wq_sb = None  # allocated after the last K matmul (reuses wk slot)

                def q_segment(strip):
                    xTq = xT_pool.tile([128, KH, 512], F32R, tag="xT")
                    for k in range(KH):
                        nc.sync.dma_start(
                            out=xTq[:, k, :],
                            in_=xqT_view[
                                :, k, strip * 512 : (strip + 1) * 512
                            ].bitcast(F32R),
                        )
                    for h in range(KH):
                        qps = mmps_pool.tile([128, 512], F32, tag="mm")
                        for k in range(KH):
                            nc.tensor.matmul(
                                qps,
                                wq_sb[:, k, h * 128 : (h + 1) * 128],
                                xTq[:, k, :],
                                start=(k == 0),
                                stop=(k == KH - 1),
                            )
                        nc.vector.tensor_copy(
                            out=q_T[:, h, strip * 512 : (strip + 1) * 512],
                            in_=qps,
                        )

                def kv_segment(chunk):
                    xT = xT_pool.tile([128, KH, 512], F32R, tag="xT")
                    for k in range(KH):
                        nc.sync.dma_start(
                            out=xT[:, k, :],
                            in_=xT_view[
                                :, k, chunk * 512 : (chunk + 1) * 512
                            ].bitcast(F32R),
                        )
                    for h in range(KH):
                        kps = mmps_pool.tile([128, 512], F32, tag="mm")
                        for k in range(KH):
                            nc.tensor.matmul(
                                kps,
                                wk_sb[:, k, h * 128 : (h + 1) * 128],
                                xT[:, k, :],
                                start=(k == 0),
                                stop=(k == KH - 1),
                            )
                        nc.vector.tensor_copy(
                            out=k_T[:, h, chunk * 512 : (chunk + 1) * 512], in_=kps
                        )
                    for t in range(4):
                        r0 = chunk * 512 + t * 128
                        for dh in range(2):
                            vps = mmps_pool.tile([128, 512], F32, tag="mm")
                            for k in range(KH):
                                nc.tensor.matmul(
                                    vps,
                                    xT[:, k, t * 128 : (t + 1) * 128],
                                    wv_sb[:, k, dh * 512 : (dh + 1) * 512],
                                    start=(k == 0),
                                    stop=(k == KH - 1),
                                )
                            vstage = vstage_pool.tile([128, 512], F32, tag="vs")
                            nc.scalar.copy(out=vstage, in_=vps)
                            nc.gpsimd.dma_start(
                                out=v_dram[r0 : r0 + 128, dh * 512 : (dh + 1) * 512],
                                in_=vstage,
                            )

                # KV chunks first (wk+wv resident), then Q strips; wq
                # reuses wk's slot once the last K matmul has retired, and its
                # DMA overlaps the last chunk's V matmuls.
                kv_segment(0)
                kv_segment(1)
                kv_segment(2)
                kv_segment(3)
                wq_sb = wpool.tile([128, KH, D], F32R, tag="w")
                for k in range(KH):
                    nc.scalar.dma_start(
                        out=wq_sb[:, k, :], in_=wq_t[:, k, :].bitcast(F32R)
                    )
                q_segment(0)
                q_segment(1)

            # ---------------- Phase 2: attention ----------------
            v_view = v_dram.rearrange("(a p) n -> p a n", p=128)  # a = s//128
            with (
                tc.tile_pool(name="oacc", bufs=1) as oacc_pool,
                tc.tile_pool(name="vsb", bufs=2) as vsb_pool,
                tc.tile_pool(name="esb", bufs=3) as esb_pool,
                tc.tile_pool(name="etsb", bufs=3) as etsb_pool,
                tc.tile_pool(name="lsb", bufs=4) as lsb_pool,
                tc.tile_pool(name="sps", bufs=2, space="PSUM") as sps_pool,
                tc.tile_pool(name="etps", bufs=2, space="PSUM") as etps_pool,
                tc.tile_pool(name="avps", bufs=2, space="PSUM") as avps_pool,
            ):
                out_acc = oacc_pool.tile([128, NB, D], F32)
                nc.sync.dma_start(out=mask_sb, in_=mask.rearrange("m p n -> p m n"))

                for j in range(4):  # 512-key blocks
                    v_sb = vsb_pool.tile([128, 4, D], F32R, tag="v")
                    nc.sync.dma_start(
                        out=v_sb, in_=v_view[:, 4 * j : 4 * j + 4, :].bitcast(F32R)
                    )
                    for i in range(2 * j, NB):  # query blocks seeing key block j
                        diag = (i // 2) == j
                        sps = sps_pool.tile([128, 512], F32, tag="s")
                        for k in range(KH):
                            nc.tensor.matmul(
                                sps,
                                q_T[:, k, i * 128 : (i + 1) * 128],
                                k_T[:, k, j * 512 : (j + 1) * 512],
                                start=(k == 0),
                                stop=(k == KH - 1),
                            )
                        if diag:  # causal boundary block
                            nc.vector.tensor_add(
                                out=sps, in0=sps, in1=mask_sb[:, i % 2, :]
                            )
                        e_sb = esb_pool.tile([128, 512], F32, tag="e")
                        lpart = lsb_pool.tile([128, 1], F32, tag="l")
                        nc.scalar.activation(
                            out=e_sb,
                            in_=sps,
                            func=mybir.ActivationFunctionType.Exp,
                            scale=SCALE,
                            accum_out=lpart,
                        )
                        if j == 0:
                            nc.vector.tensor_copy(out=l_acc[:, i : i + 1], in_=lpart)
                        else:
                            nc.vector.tensor_add(
                                out=l_acc[:, i : i + 1],
                                in0=l_acc[:, i : i + 1],
                                in1=lpart,
                            )
                        # E^T: 4 PE transposes into one PSUM tile, one copy out
                        etp = etps_pool.tile([128, 512], F32, tag="et")
                        for jj in range(4):
                            nc.tensor.transpose(
                                etp[:, jj * 128 : (jj + 1) * 128],
                                e_sb[:, jj * 128 : (jj + 1) * 128],
                                ident,
                            )
                        et = etsb_pool.tile([128, 512], F32R, tag="ets")
                        nc.vector.tensor_copy(out=et, in_=etp)
                        avps = avps_pool.tile([128, D], F32, tag="av")
                        for jj in range(4):
                            for dh in range(2):
                                nc.tensor.matmul(
                                    avps[:, dh * 512 : (dh + 1) * 512],
                                    et[:, jj * 128 : (jj + 1) * 128],
                                    v_sb[:, jj, dh * 512 : (dh + 1) * 512],
                                    start=(jj == 0),
                                    stop=(jj == 3),
                                )
                        if j == 0:
                            nc.vector.tensor_copy(out=out_acc[:, i, :], in_=avps)
                        else:
                            nc.vector.tensor_add(
                                out=out_acc[:, i, :], in0=out_acc[:, i, :], in1=avps
                            )
                        if diag:
                            # block i is complete: normalize and write out now
                            rinv = lsb_pool.tile([128, 1], F32, tag="r")
                            nc.vector.reciprocal(out=rinv, in_=l_acc[:, i : i + 1])
                            nc.vector.tensor_scalar_mul(
                                out=out_acc[:, i, :],
                                in0=out_acc[:, i, :],
                                scalar1=rinv,
                            )
                            nc.gpsimd.dma_start(
                                out=y[i * 128 : (i + 1) * 128, :],
                                in_=out_acc[:, i, :],
                            )

    return nc


def _get_nc(finalize=True):
    key = "nc_fin" if finalize else "nc_raw"
    if key not in _CACHE:
        nc = _build_nc()
        if finalize:
            nc.finalize()
        _CACHE[key] = nc
    return _CACHE[key]


def make_in_maps(x, Wq, Wk, Wv):
    ident = np.eye(128, dtype=np.float32)
    p = np.arange(128)[:, None]
    c = np.arange(512)[None, :]
    in_maps = []
    for core in range(8):
        b, par = core // 2, core % 2
        # mask[0]: boundary block for even local i; mask[1]: odd local i
        m0 = np.where(c <= p + par * 128, 0.0, NEG).astype(np.float32)
        m1 = np.where(c <= 256 + par * 128 + p, 0.0, NEG).astype(np.float32)
        xb = np.asarray(x[b], dtype=np.float32)
        xq = xb.reshape(16, 128, D)[par::2].reshape(NB * 128, D)
        in_maps.append(
            {
                "x_T": np.ascontiguousarray(xb.T),
                "x_qT": np.ascontiguousarray(xq.T),
                "wq": np.ascontiguousarray(Wq, dtype=np.float32),
                "wk": np.ascontiguousarray(Wk, dtype=np.float32),
                "wv": np.ascontiguousarray(Wv, dtype=np.float32),
                "mask": np.stack([m0, m1]),
                "ident": ident,
            }
        )
    return in_maps


def assemble_out(results):
    out = np.empty((B, S, D), dtype=np.float32)
    o4 = out.reshape(B, 16, 128, D)
    for core in range(8):
        b, par = core // 2, core % 2
        o4[b, par::2] = results[core]["y"].reshape(NB, 128, D)
    return out


def kernel(x, Wq, Wk, Wv):
    global LAST_RESULT
    from concourse.bass_utils import run_bass_kernel_spmd

    nc = _get_nc(finalize=True)
    in_maps = make_in_maps(x, Wq, Wk, Wv)
    res = run_bass_kernel_spmd(nc, in_maps, core_ids=list(range(8)))
    LAST_RESULT = res
    return assemble_out(res.results)
